# revision 1
# baseline (speedup 1.0000x reference)
"""Trainium2 Bass kernel for BCNet-style fused block.

Reference computation (per batch b):
    v_ = relu(v @ Wv.T + bv)            # [B, NO, H]
    q_ = relu(q @ Wq.T + bq)            # [B, Q,  H]
    qw = einsum("bqh,q->bh", q_, wh)    # [B, H]
    logits = v_ * qw[:, None, :] + bh   # [B, NO, H]
    out = logits @ W2.T + b2            # [B, NO, VD]

Strategy: pure data parallel over batch (16 per core x 8 cores), weights
replicated. All matmuls run in bf16 with fp32 PSUM accumulation; weights /
activations are pre-cast and pre-transposed on host so the device streams
them in matmul-native layouts with no on-chip transposes.

Per-core dataflow (H or VD on the partition dim throughout):
  A: q_T = relu(WqT.T @ qT + bq)    -> *wh -> segment-reduce over Q -> qw_T
  B: v_T = relu(WvT.T @ vT + bv)    -> logits_T = v_T * qw_T (broadcast)
  C: out_T = W2T.T @ logits_T + b2eff  (bh folded into b2eff on host)
Output is produced transposed [VD, rows]; host transposes back.

Scheduling notes (engines execute their streams in order; DMA transfers are
effectively serialized at ~350GB/s, dma_start dispatch ~0.6us per queue):
- Matmul loops run k-outer over blocks of concurrent PSUM groups so each
  arriving weight chunk unlocks work in every in-flight group.
- Weights load as a few large DMAs (one per column block, all k-tiles in
  one 3D access pattern), emitted in PE consumption order.
- The first B block (m 0-3, both n halves) is issued BEFORE phase A: it
  only needs vT + the first WvT column block, so ~15us of real PE work
  runs while the WqT stream is still on the bus. Its evictions are
  split: the ACT relus run immediately (freeing the PSUM banks so phase
  A can use 8-group halves); the qw multiplies are deferred to after A.
- Bus order is hand-paced to PE consumption: vT/WvT k-chunks first,
  consts and qT (needed only by evictions / phase A) after them.
- ~100 tiny warmup matmuls on a zeroed tile fill the initial DMA wait so
  the PE clock (HAM) is already un-throttled when the real stream starts.
"""

import os
import sys

import numpy as np

for _p in ("/opt/trn_rl_repo", "/root/.axon_site/_ro/trn_rl_repo"):
    if os.path.isdir(_p) and _p not in sys.path:
        sys.path.insert(0, _p)

import ml_dtypes

import concourse.bacc as bacc
import concourse.bass as bass
import concourse.mybir as mybir
import concourse.tile as tile
from concourse.bass_utils import run_bass_kernel_spmd

B, NO, Q = 128, 36, 14
VD, QD, H = 2048, 1024, 2048
NCORES = 8
BS = B // NCORES          # 16 batches per core
NROW = BS * NO            # 576 v-rows per core
QROW = BS * Q             # 224 q-rows per core
P = 128
NT = 288                  # n-tile for matmuls 1/3 (2 tiles of 8 batches * 36)
NN = NROW // NT           # 2
BPT = NT // NO            # 8 batches per n-tile
KV = VD // P              # 16 contraction tiles for matmul 1
KQ = QD // P              # 8  contraction tiles for matmul 2
MH = H // P               # 16 output h-tiles
KH = H // P               # 16 contraction tiles for matmul 3
MV = VD // P              # 16 output vd-tiles

F32 = mybir.dt.float32
BF16 = mybir.dt.bfloat16
BF16_NP = ml_dtypes.bfloat16


def _build_program(opts=None):
    o = dict(
        wq_split=2,   # column blocks for WqT (phase-A pacing granularity)
        wv_split=4,   # column blocks for WvT (must match phase-B m-blocks of 4)
        w2_split=4,   # column blocks for W2T (16KB/partition slot, matches wv)
        warmup=100,   # PE warmup matmuls before the first real matmul
        out_split=True,   # one output DMA per (m, n) instead of per m
        wq_eng="sync",    # queue for the WqT stream
        out_eng="sync",   # issuing engine for output DMAs
        wv0_chunks=8,     # k-chunks for the first WvT column block
        tail_split=True,  # half-width final output group (shorter tail)
        b0_first=True,  # issue B-block0 (m0-3, n0) before phase A
        psum_bufs=8,
    )
    if opts:
        o.update(opts)

    nc = bacc.Bacc("TRN2", target_bir_lowering=False, debug=False, num_devices=NCORES)

    vT = nc.dram_tensor("vT", [P, NN * KV * NT], BF16, kind="ExternalInput").ap()
    qT = nc.dram_tensor("qT", [P, KQ * QROW], BF16, kind="ExternalInput").ap()
    WvT = nc.dram_tensor("WvT", [VD, H], BF16, kind="ExternalInput").ap()
    WqT = nc.dram_tensor("WqT", [QD, H], BF16, kind="ExternalInput").ap()
    W2T = nc.dram_tensor("W2T", [H, VD], BF16, kind="ExternalInput").ap()
    constC = nc.dram_tensor("constC", [P, 3 * 16 + QROW], F32,
                            kind="ExternalInput").ap()
    outT = nc.dram_tensor("outT", [VD, NROW], F32, kind="ExternalOutput").ap()

    # DRAM views with k-tiles split out
    qT_r = qT.rearrange("p (k c) -> p k c", k=KQ)
    vT_r = vT.rearrange("p (n k c) -> p n k c", n=NN, k=KV)
    WqT_r = WqT.rearrange("(k p) c -> p k c", p=P)
    WvT_r = WvT.rearrange("(k p) c -> p k c", p=P)
    W2T_r = W2T.rearrange("(k p) c -> p k c", p=P)

    with tile.TileContext(nc) as tc:
        from contextlib import ExitStack

        with ExitStack() as ctx:
            wpool = ctx.enter_context(tc.tile_pool(name="weights", bufs=8))
            apool = ctx.enter_context(tc.tile_pool(name="acts", bufs=1))
            lpool = ctx.enter_context(tc.tile_pool(name="logits", bufs=MH))
            qwpool = ctx.enter_context(tc.tile_pool(name="qw", bufs=MH))
            const = ctx.enter_context(tc.tile_pool(name="const", bufs=1))
            stage = ctx.enter_context(tc.tile_pool(name="stage", bufs=6))
            b0pool = ctx.enter_context(tc.tile_pool(name="b0stage", bufs=8))
            psum = ctx.enter_context(
                tc.tile_pool(name="psum", bufs=o["psum_bufs"], space="PSUM"))

            # Consts packed into one DMA: bv | bq | b2eff | wh
            cst = const.tile([P, 3 * 16 + QROW], F32)

            def dma_cst():
                nc.sync.dma_start(out=cst[:], in_=constC)
            bv_sb = cst[:, 0:16]
            bq_sb = cst[:, 16:32]
            b2_sb = cst[:, 32:48]
            wh_sb = cst[:, 48:48 + QROW]

            if o["warmup"]:
                wup = stage.tile([P, 64], BF16, tag="wup", name="wup")
                nc.vector.memset(wup[:], 0.0)
                wps = psum.tile([64, 64], F32, tag="ps", name="pswarm")
                for _ in range(o["warmup"]):
                    nc.tensor.matmul(wps[:], lhsT=wup[:, 0:64], rhs=wup[:],
                                     start=True, stop=True)

            # SBUF tiles (allocation order is not DMA order)
            vtn = [apool.tile([P, KV, NT], BF16, name=f"vt{n}") for n in range(NN)]
            qt_all = apool.tile([P, KQ, QROW], BF16)
            wq_cb = H // o["wq_split"]
            wqts = [wpool.tile([P, KQ, wq_cb], BF16, tag="w", name=f"wq{s}")
                    for s in range(o["wq_split"])]
            wv_cb = H // o["wv_split"]
            wvts = [wpool.tile([P, KV, wv_cb], BF16, tag="w", name=f"wv{s}")
                    for s in range(o["wv_split"])]
            w2_cb = VD // o["w2_split"]
            w2ts = [wpool.tile([P, KH, w2_cb], BF16, tag="w", name=f"w2{s}")
                    for s in range(o["w2_split"])]

            def dma_vt(n, k0=0, k1=KV):
                nc.sync.dma_start(out=vtn[n][:, k0:k1, :], in_=vT_r[:, n, k0:k1, :])

            def dma_qt():
                nc.sync.dma_start(out=qt_all[:], in_=qT_r)

            def dma_wq(s, k0=0, k1=KQ):
                e = {"sync": nc.sync, "gpsimd": nc.gpsimd,
                     "scalar": nc.scalar}[o["wq_eng"]]
                e.dma_start(out=wqts[s][:, k0:k1, :],
                            in_=WqT_r[:, k0:k1, s * wq_cb:(s + 1) * wq_cb])

            def dma_wv(s, k0=0, k1=KV):
                nc.sync.dma_start(out=wvts[s][:, k0:k1, :],
                                  in_=WvT_r[:, k0:k1, s * wv_cb:(s + 1) * wv_cb])

            def dma_w2(s):
                nc.sync.dma_start(out=w2ts[s][:],
                                  in_=W2T_r[:, :, s * w2_cb:(s + 1) * w2_cb])

            # DMA emission order == HWDGE dispatch order == transfer order.
            # Hand-paced: each chunk lands just before the PE stream needs it
            # (PE order: warmup, B-b0 (m0-3, n0 then n1, ACT-only evictions),
            #  A halves, deferred b0 qw-multiplies, B blocks m4-15, C).
            if o["b0_first"]:
                ck = KV // o["wv0_chunks"]
                dma_vt(0, 0, 8)
                for c in range(0, 8 // ck):
                    dma_wv(0, c * ck, (c + 1) * ck)
                dma_vt(0, 8, 16)
                for c in range(8 // ck, KV // ck):
                    dma_wv(0, c * ck, (c + 1) * ck)
                dma_cst()
                dma_vt(1, 0, 8)
                dma_vt(1, 8, 16)
                dma_qt()
                dma_wq(0, 0, 4)
                dma_wq(0, 4, 8)
                dma_wq(1, 0, 4)
                dma_wq(1, 4, 8)
                dma_wv(1, 0, 8)
                dma_wv(1, 8, 16)
            else:
                dma_cst()
                dma_qt()
                dma_vt(0, 0, 8)
                dma_wv(0, 0, 4)
                dma_vt(0, 8, 16)
                dma_wv(0, 4, 8)
                dma_wv(0, 8, 12)
                dma_wv(0, 12, 16)
                dma_wq(0, 0, 4)
                dma_wq(0, 4, 8)
                dma_wq(1, 0, 4)
                dma_wq(1, 4, 8)
                dma_vt(1)
                dma_wv(1)
            for s in range(2, o["wv_split"]):
                dma_wv(s)
            for s in range(o["w2_split"]):
                dma_w2(s)

            def wq_lhsT(k, m):
                s, r = divmod(m * P, wq_cb)
                return wqts[s][:, k, r:r + P]

            def wv_lhsT(k, m):
                s, r = divmod(m * P, wv_cb)
                return wvts[s][:, k, r:r + P]

            def w2_lhsT(k, m):
                s, r = divmod(m * P, w2_cb)
                return w2ts[s][:, k, r:r + P]

            lts = [None] * MH
            qwts = [None] * MH

            def b_matmuls(groups, pss):
                for k in range(KV):
                    for (m, n) in groups:
                        nc.tensor.matmul(
                            pss[(m, n)][:], lhsT=wv_lhsT(k, m),
                            rhs=vtn[n][:, k, :],
                            start=(k == 0), stop=(k == KV - 1))

            def b_evict(m, n, ps):
                vs = stage.tile([P, NT], F32, tag="vstage", name=f"vs{m}_{n}")
                nc.scalar.activation(vs[:], ps[:],
                                     mybir.ActivationFunctionType.Relu,
                                     bias=bv_sb[:, m:m + 1])
                qb = qwts[m][:, n * BPT:(n + 1) * BPT].to_broadcast([P, BPT, NO])
                nc.vector.tensor_mul(
                    lts[m][:, n * NT:(n + 1) * NT].rearrange(
                        "p (b o) -> p b o", b=BPT),
                    vs.rearrange("p (b o) -> p b o", b=BPT), qb)

            def a_block(ms):
                pss = {m: psum.tile([P, QROW], F32, tag="ps", name=f"psA{m}")
                       for m in ms}
                for k in range(KQ):
                    for m in ms:
                        nc.tensor.matmul(
                            pss[m][:], lhsT=wq_lhsT(k, m), rhs=qt_all[:, k, :],
                            start=(k == 0), stop=(k == KQ - 1))
                for m in ms:
                    qs = stage.tile([P, QROW], F32, tag="qstage", name=f"qs{m}")
                    nc.scalar.activation(qs[:], pss[m][:],
                                         mybir.ActivationFunctionType.Relu,
                                         bias=bq_sb[:, m:m + 1])
                    qp = stage.tile([P, QROW], F32, tag="qstage", name=f"qp{m}")
                    nc.vector.tensor_mul(qp[:], qs[:], wh_sb)
                    qw = qwpool.tile([P, BS], F32, tag="qw", name=f"qw{m}")
                    nc.vector.tensor_reduce(
                        qw[:], qp.rearrange("p (b q) -> p b q", b=BS),
                        axis=mybir.AxisListType.X, op=mybir.AluOpType.add)
                    qwts[m] = qw

            if o["b0_first"]:
                # B-block0 (m0-3), n=0 then n=1: matmuls + ACT relu now (the
                # relu frees the PSUM banks); the qw multiplies are deferred
                # until phase A has produced qw. This front-loads 15.4us of
                # real PE work that only needs vT + the first WvT column
                # block, while the WqT stream is still on the bus.
                for m in range(4):
                    lts[m] = lpool.tile([P, NROW], BF16, tag="lt", name=f"lt{m}")
                b0_vs = {}
                for n in range(NN):
                    g0 = [(m, n) for m in range(4)]
                    pss0 = {(m, n): psum.tile([P, NT], F32, tag="ps",
                                              name=f"psB{m}_{n}")
                            for m in range(4)}
                    b_matmuls(g0, pss0)
                    for m in range(4):
                        vs = b0pool.tile([P, NT], F32, tag="b0s",
                                         name=f"b0vs{m}_{n}")
                        nc.scalar.activation(vs[:], pss0[(m, n)][:],
                                             mybir.ActivationFunctionType.Relu,
                                             bias=bv_sb[:, m:m + 1])
                        b0_vs[(m, n)] = vs
                # Phase A in halves (b0's banks were released by the relus).
                for half in range(2):
                    a_block(list(range(half * 8, half * 8 + 8)))
                for (m, n), vs in b0_vs.items():
                    qb = qwts[m][:, n * BPT:(n + 1) * BPT].to_broadcast(
                        [P, BPT, NO])
                    nc.vector.tensor_mul(
                        lts[m][:, n * NT:(n + 1) * NT].rearrange(
                            "p (b o) -> p b o", b=BPT),
                        vs.rearrange("p (b o) -> p b o", b=BPT), qb)
                rest_blocks = [list(range(4, 8)), list(range(8, 12)),
                               list(range(12, 16))]
            else:
                for half in range(2):
                    a_block(list(range(half * 8, half * 8 + 8)))
                rest_blocks = [list(range(0, 4)), list(range(4, 8)),
                               list(range(8, 12)), list(range(12, 16))]

            for ms in rest_blocks:
                for m in ms:
                    lts[m] = lpool.tile([P, NROW], BF16, tag="lt", name=f"lt{m}")
                groups = [(m, n) for m in ms for n in range(NN)]
                pss = {(m, n): psum.tile([P, NT], F32, tag="ps", name=f"psB{m}_{n}")
                       for (m, n) in groups}
                b_matmuls(groups, pss)
                for (m, n) in groups:
                    b_evict(m, n, pss[(m, n)])

            # ---- Phase C: out_T[vd, n] = W2 @ logits + b2eff
            eng_out = {"sync": nc.sync, "scalar": nc.scalar}[o["out_eng"]]
            for m in range(MV):
                os_ = stage.tile([P, NROW], F32, tag="ostage", name=f"os{m}")
                for n in range(NN):
                    # Split the very last group in half so the kernel-tail
                    # evict->DMA chain runs on a half-width tile.
                    last = (m == MV - 1 and n == NN - 1)
                    nsub = 2 if (last and o["tail_split"]) else 1
                    w = NT // nsub
                    for h in range(nsub):
                        c0 = n * NT + h * w
                        ps = psum.tile([P, w], F32, tag="ps",
                                       name=f"psC{m}_{n}_{h}")
                        for k in range(KH):
                            nc.tensor.matmul(
                                ps[:], lhsT=w2_lhsT(k, m),
                                rhs=lts[k][:, c0:c0 + w],
                                start=(k == 0), stop=(k == KH - 1))
                        nc.scalar.activation(os_[:, c0:c0 + w], ps[:],
                                             mybir.ActivationFunctionType.Identity,
                                             bias=b2_sb[:, m:m + 1])
                        if o["out_split"]:
                            eng_out.dma_start(
                                out=outT[m * P:(m + 1) * P, c0:c0 + w],
                                in_=os_[:, c0:c0 + w])
                if not o["out_split"]:
                    eng_out.dma_start(
                        out=outT[m * P:(m + 1) * P, :], in_=os_[:])

    nc.compile()
    return nc


_NC_CACHE = {}


def get_program(opts=None):
    key = tuple(sorted(opts.items())) if opts else ()
    if key not in _NC_CACHE:
        _NC_CACHE[key] = _build_program(opts)
    return _NC_CACHE[key]


def make_in_maps(v, q, Wv, bv, Wq, bq, wh, bh, W2, b2):
    """Host-side prep: shard batch, pre-transpose, pre-cast."""
    WvT = np.ascontiguousarray(Wv.astype(BF16_NP).T)           # [VD, H]
    WqT = np.ascontiguousarray(Wq.astype(BF16_NP).T)           # [QD, H]
    W2T = np.ascontiguousarray(W2.astype(BF16_NP).T)           # [H, VD]
    b2eff = (b2.astype(np.float64)
             + float(bh) * W2.astype(np.float64).sum(axis=1)).astype(np.float32)
    constC = np.zeros((P, 3 * 16 + QROW), np.float32)
    constC[:, 0:16] = bv.astype(np.float32).reshape(MH, P).T
    constC[:, 16:32] = bq.astype(np.float32).reshape(MH, P).T
    constC[:, 32:48] = b2eff.reshape(MV, P).T
    constC[:, 48:] = np.tile(wh.astype(np.float32), BS)[None, :]

    in_maps = []
    for c in range(NCORES):
        b0 = c * BS
        v_sh = v[b0:b0 + BS].reshape(NROW, VD).astype(BF16_NP)
        q_sh = q[b0:b0 + BS].reshape(QROW, QD).astype(BF16_NP)
        # vT: [P, n, k, c] flattened; qT: [P, k, c] flattened (k-major rows
        # contiguous per partition for single-descriptor DMAs)
        vT_c = (v_sh.T.reshape(KV, P, NN, NT).transpose(1, 2, 0, 3)
                .reshape(P, NN * KV * NT))
        qT_c = q_sh.T.reshape(KQ, P, QROW).transpose(1, 0, 2).reshape(P, KQ * QROW)
        in_maps.append({
            "vT": np.ascontiguousarray(vT_c),
            "qT": np.ascontiguousarray(qT_c),
            "WvT": WvT, "WqT": WqT, "W2T": W2T,
            "constC": constC,
        })
    return in_maps


def assemble_output(results):
    outs = []
    for c in range(NCORES):
        outT = results[c]["outT"]                      # [VD, NROW] f32
        outs.append(np.ascontiguousarray(outT.T).reshape(BS, NO, VD))
    return np.concatenate(outs, axis=0)


def kernel(v, q, Wv, bv, Wq, bq, wh, bh, W2, b2, **_unused):
    v, q, Wv, bv, Wq, bq, wh, bh, W2, b2 = (
        np.asarray(x) for x in (v, q, Wv, bv, Wq, bq, wh, bh, W2, b2))
    nc = get_program()
    in_maps = make_in_maps(v, q, Wv, bv, Wq, bq, wh, bh, W2, b2)
    res = run_bass_kernel_spmd(nc, in_maps, list(range(NCORES)))
    return assemble_output(res.results)



# revision 2
# speedup vs baseline: 1.0028x; 1.0028x over previous
"""Trainium2 Bass kernel for BCNet-style fused block — fp8 DoubleRow version.

Reference computation (per batch b):
    v_ = relu(v @ Wv.T + bv)            # [B, NO, H]
    q_ = relu(q @ Wq.T + bq)            # [B, Q,  H]
    qw = einsum("bqh,q->bh", q_, wh)    # [B, H]
    logits = v_ * qw[:, None, :] + bh   # [B, NO, H]
    out = logits @ W2.T + b2            # [B, NO, VD]

Strategy: pure data parallel over batch (16 per core x 8 cores), weights
replicated. All three matmuls run as fp8(e4m3) DoubleRow matmuls (0.5
cycles/output-row, 256-deep contraction per instruction = 4x bf16 FLOP
rate) with a 3-pass error-compensation scheme:

    x @ W ~= xh@Wh + xl@Wh + xh@Wl,   xh = fp8(x), xl = fp8(x - xh)

The residuals are stored UNSCALED (partly subnormal fp8 — verified exact
on hardware), so all three passes share one PSUM accumulation chain and
the eviction stays a single activation read, exactly like a bf16 kernel.
Measured per-matmul error of this scheme is ~1.3e-3 (better than bf16).

Scale folding: weights are pre-scaled x32 on host so their values sit in
e4m3's normal range; logits are pre-scaled x8 by folding 8 into wh. Both
scales fold out for free in the eviction activations (out = f(in*scale+b)).

Per-core dataflow (H or VD on the partition dim throughout):
  A: q_T = relu(WqT.T @ qT + bq)  -> *(wh*8) -> segment-reduce -> qw_T
  B: v_T = relu(WvT.T @ vT + bv)  -> lg = v_T * qw_T (f32) -> split into
     lts_hi = fp8(lg), lts_lo = fp8(lg - lts_hi)
  C: out_T = (W2T*32).T @ [lts_hi/lo] / 256 + b2eff
Output is produced transposed [VD, rows]; host transposes back.

Schedule: one global block pipeline with lag-1 evictions (PSUM banks
recycle while the PE streams; each PE gap would also reset the p-state
ramp). PE stream: warmup | B m0-7 ("b0", blocks of 4m x one 192-wide
n-chunk — v arrives n-chunk-major so the first block starts after ~1MB)
| A m0-15 (4-m blocks) | B m8-15 | C (per (m,n) groups). Pass order is
(hh, lh, hl) everywhere: weight-lo arrays arrive before act-lo arrays,
and in C the logits-lo tiles are produced last. b0's qw-multiplies are
deferred until A produces qw. DMA emission is hand-paced to this
consumption order (the serial input stream is ~67us vs ~101us of PE
work). Output DMAs issue from rotating gpsimd/sync queues so their
~625ns HWDGE dispatches never serialize against the input stream or the
ACT eviction queue; the final m-tile is split fine across queues to
shorten the tail evict->dispatch->transfer chain.
"""

import os
import sys

import numpy as np

for _p in ("/opt/trn_rl_repo", "/root/.axon_site/_ro/trn_rl_repo"):
    if os.path.isdir(_p) and _p not in sys.path:
        sys.path.insert(0, _p)

import ml_dtypes

import concourse.bacc as bacc
import concourse.bass as bass
import concourse.mybir as mybir
import concourse.tile as tile
from concourse.bass_utils import run_bass_kernel_spmd

B, NO, Q = 128, 36, 14
VD, QD, H = 2048, 1024, 2048
NCORES = 8
BS = B // NCORES          # 16 batches per core
NROW = BS * NO            # 576 v-rows per core
QROW = BS * Q             # 224 q-rows per core
P = 128
NT = 192                  # n-chunk for matmuls 1/3 (2*NT <= 512 moving limit)
NN = NROW // NT           # 3 n-chunks
KC1 = VD // 256           # 8 DoubleRow k-steps for matmul 1
KC2 = QD // 256           # 4 for matmul 2
KC3 = H // 256            # 8 for matmul 3
MH = H // P               # 16 output h-tiles
MV = VD // P              # 16 output vd-tiles
SW = 32.0                 # weight pre-scale (host)
SL = 8.0                  # logits pre-scale (folded into wh)

F32 = mybir.dt.float32
BF16 = mybir.dt.bfloat16
F8 = mybir.dt.float8e4
F8NP = ml_dtypes.float8_e4m3
DRM = mybir.MatmulPerfMode.DoubleRow
RELU = mybir.ActivationFunctionType.Relu
IDENT = mybir.ActivationFunctionType.Identity


def _build_program(opts=None):
    o = dict(
        warmup=88,
        out_engs=("gpsimd", "sync"),
        tail_engs=("gpsimd", "sync", "gpsimd", "sync"),
    )
    if opts:
        o.update(opts)

    nc = bacc.Bacc("TRN2", target_bir_lowering=False, debug=False, num_devices=NCORES)

    def din(name, free):
        return nc.dram_tensor(name, [P, free], F8, kind="ExternalInput").ap()

    vh_d, vl_d = din("vh", NN * KC1 * 2 * NT), din("vl", NN * KC1 * 2 * NT)
    qh_d, ql_d = din("qh", KC2 * 2 * QROW), din("ql", KC2 * 2 * QROW)
    wvh_d, wvl_d = din("wvh", KC1 * 2 * H), din("wvl", KC1 * 2 * H)
    wqh_d, wql_d = din("wqh", KC2 * 2 * H), din("wql", KC2 * 2 * H)
    w2h_d, w2l_d = din("w2h", KC3 * 2 * VD), din("w2l", KC3 * 2 * VD)
    constC = nc.dram_tensor("constC", [P, 3 * 16 + QROW], F32,
                            kind="ExternalInput").ap()
    outT = nc.dram_tensor("outT", [VD, NROW], F32, kind="ExternalOutput").ap()

    def rr(ap, c):
        return ap.rearrange("p (c j n) -> p c j n", c=c, j=2)

    # v is n-chunk-major: [p, nb, c, j, 192]
    vh_r = vh_d.rearrange("p (b c j n) -> p b c j n", b=NN, c=KC1, j=2)
    vl_r = vl_d.rearrange("p (b c j n) -> p b c j n", b=NN, c=KC1, j=2)
    qh_r, ql_r = rr(qh_d, KC2), rr(ql_d, KC2)
    wvh_r, wvl_r = rr(wvh_d, KC1), rr(wvl_d, KC1)
    wqh_r, wql_r = rr(wqh_d, KC2), rr(wql_d, KC2)
    w2h_r, w2l_r = rr(w2h_d, KC3), rr(w2l_d, KC3)

    with tile.TileContext(nc) as tc:
        from contextlib import ExitStack

        with ExitStack() as ctx:
            wpool = ctx.enter_context(tc.tile_pool(name="weights", bufs=16))
            apool = ctx.enter_context(tc.tile_pool(name="acts", bufs=1))
            qwpool = ctx.enter_context(tc.tile_pool(name="qw", bufs=MH))
            const = ctx.enter_context(tc.tile_pool(name="const", bufs=1))
            stage = ctx.enter_context(tc.tile_pool(name="stage", bufs=3))
            ospool = ctx.enter_context(tc.tile_pool(name="ostage", bufs=2))
            b0pool = ctx.enter_context(tc.tile_pool(name="b0stage", bufs=1))
            psum = ctx.enter_context(
                tc.tile_pool(name="psum", bufs=8, space="PSUM"))

            cst = const.tile([P, 3 * 16 + QROW], F32)
            bv_sb = cst[:, 0:16]
            bq_sb = cst[:, 16:32]
            b2_sb = cst[:, 32:48]
            whx_sb = cst[:, 48:48 + QROW]

            if o["warmup"]:
                wup = stage.tile([P, 64], BF16, tag="wup", name="wup")
                nc.vector.memset(wup[:], 0.0)
                wps = psum.tile([64, 64], F32, tag="ps", name="pswarm")
                for _ in range(o["warmup"]):
                    nc.tensor.matmul(wps[:], lhsT=wup[:, 0:64], rhs=wup[:],
                                     start=True, stop=True)

            _psn = [0]

            def ps_alloc(w):
                _psn[0] += 1
                return psum.tile([P, w], F32, tag="ps", name=f"ps{_psn[0]}")[:]

            # Weight pool: 20 tiles in a 16-slot ring; w2h2/3 reuse wvh0/1
            # (freed when b0 = B m0-7 ends) and w2l0/1 reuse wvl0/1.
            wvh = [wpool.tile([P, KC1, 2, 512], F8, tag="w", name=f"wvh{s}")
                   for s in range(4)]
            wvl = [wpool.tile([P, KC1, 2, 512], F8, tag="w", name=f"wvl{s}")
                   for s in range(4)]
            wqh = [wpool.tile([P, KC2, 2, 1024], F8, tag="w", name=f"wqh{s}")
                   for s in range(2)]
            wql = [wpool.tile([P, KC2, 2, 1024], F8, tag="w", name=f"wql{s}")
                   for s in range(2)]
            w2h = [wpool.tile([P, KC3, 2, 512], F8, tag="w", name=f"w2h{s}")
                   for s in range(4)]
            w2l = [wpool.tile([P, KC3, 2, 512], F8, tag="w", name=f"w2l{s}")
                   for s in range(4)]

            vh_s = apool.tile([P, NN, KC1, 2, NT], F8)
            vl_s = apool.tile([P, NN, KC1, 2, NT], F8)
            qh_s = apool.tile([P, KC2, 2, QROW], F8)
            ql_s = apool.tile([P, KC2, 2, QROW], F8)
            lts_hi = apool.tile([P, MH, NROW], F8)
            lts_lo = apool.tile([P, MH, NROW], F8)

            def dma(sb, dr):
                nc.sync.dma_start(out=sb, in_=dr)

            # DMA emission order == transfer order, paced to PE consumption.
            VC = KC1 * 2 * NT           # bytes per v n-chunk per partition
            dma(vh_s[:, 0], vh_d[:, 0:VC])
            dma(wvh[0][:, 0:4], wvh_r[:, 0:4, :, 0:512])
            dma(wvh[0][:, 4:8], wvh_r[:, 4:8, :, 0:512])
            dma(wvl[0][:, 0:4], wvl_r[:, 0:4, :, 0:512])
            dma(vl_s[:, 0], vl_d[:, 0:VC])
            dma(wvl[0][:, 4:8], wvl_r[:, 4:8, :, 0:512])
            dma(cst[:], constC)
            dma(vh_s[:, 1], vh_d[:, VC:2 * VC])
            dma(vl_s[:, 1], vl_d[:, VC:2 * VC])
            dma(vh_s[:, 2], vh_d[:, 2 * VC:3 * VC])
            dma(vl_s[:, 2], vl_d[:, 2 * VC:3 * VC])
            dma(wvh[1][:], wvh_r[:, :, :, 512:1024])
            dma(wvl[1][:], wvl_r[:, :, :, 512:1024])
            dma(qh_s[:], qh_d)
            dma(wqh[0][:], wqh_r[:, :, :, 0:1024])
            dma(wql[0][:], wql_r[:, :, :, 0:1024])
            dma(ql_s[:], ql_d)
            dma(wqh[1][:], wqh_r[:, :, :, 1024:2048])
            dma(wql[1][:], wql_r[:, :, :, 1024:2048])
            for s in range(2, 4):
                dma(wvh[s][:], wvh_r[:, :, :, s * 512:(s + 1) * 512])
                dma(wvl[s][:], wvl_r[:, :, :, s * 512:(s + 1) * 512])
            dma(w2h[0][:], w2h_r[:, :, :, 0:512])
            dma(w2l[0][:], w2l_r[:, :, :, 0:512])
            dma(w2h[1][:], w2h_r[:, :, :, 512:1024])
            dma(w2l[1][:], w2l_r[:, :, :, 512:1024])
            dma(w2h[2][:], w2h_r[:, :, :, 1024:1536])
            dma(w2h[3][:], w2h_r[:, :, :, 1536:2048])
            dma(w2l[2][:], w2l_r[:, :, :, 1024:1536])
            dma(w2l[3][:], w2l_r[:, :, :, 1536:2048])

            def w_lhsT(tiles, cb, c, m):
                s, r = divmod(m * P, cb)
                return tiles[s][:, c, :, r:r + P]

            qwts = [None] * MH

            # ---- global block pipeline: emit matmuls of block i, then the
            # evictions of block i-1 (PSUM recycles during block i).
            _pending = [None]

            def pipe(mm_fn, ev_fn):
                mm_fn()
                if _pending[0] is not None:
                    _pending[0]()
                _pending[0] = ev_fn

            def pipe_flush():
                if _pending[0] is not None:
                    _pending[0]()
                _pending[0] = None

            # ---- phase B pieces (blocks of 4 m-tiles x one n-chunk)
            def b_block(ms, nb, vs_of):
                pss = {m: ps_alloc(NT) for m in ms}

                def mm():
                    for pi, (wt, vt) in enumerate(
                            [(wvh, vh_s), (wvl, vh_s), (wvh, vl_s)]):
                        for c in range(KC1):
                            for m in ms:
                                nc.tensor.matmul(
                                    pss[m],
                                    lhsT=w_lhsT(wt, 512, c, m),
                                    rhs=vt[:, nb, c, :, :],
                                    start=(pi == 0 and c == 0),
                                    stop=(pi == 2 and c == KC1 - 1),
                                    perf_mode=DRM)

                def ev():
                    for m in ms:
                        nc.scalar.activation(
                            vs_of(m)[:, nb * NT:(nb + 1) * NT], pss[m],
                            RELU, bias=bv_sb[:, m:m + 1], scale=1.0 / SW)

                return mm, ev

            def b_finish(m, vs, sub_eng=None, half=None):
                # half=0/1: batch-aligned 288-col halves (8 batches each)
                lg_t = stage.tile([P, NROW], F32, tag="lg", name=f"lg{m}")
                halves = (0, 1) if half is None else (half,)
                for hf in halves:
                    sl = slice(hf * 288, (hf + 1) * 288)
                    lg = lg_t[:, sl]
                    qb = qwts[m][:, hf * 8:(hf + 1) * 8].to_broadcast(
                        [P, 8, NO])
                    nc.vector.tensor_mul(
                        lg.rearrange("p (b o) -> p b o", b=8),
                        vs[:, sl].rearrange("p (b o) -> p b o", b=8), qb)
                    nc.scalar.copy(lts_hi[:, m, sl], lg)
                    (sub_eng or nc.vector).tensor_sub(
                        lts_lo[:, m, sl], lg, lts_hi[:, m, sl])

            def a_block(ms):
                pss = {m: ps_alloc(QROW) for m in ms}

                def mm():
                    for pi, (wt, qt) in enumerate(
                            [(wqh, qh_s), (wql, qh_s), (wqh, ql_s)]):
                        for c in range(KC2):
                            for m in ms:
                                nc.tensor.matmul(
                                    pss[m],
                                    lhsT=w_lhsT(wt, 1024, c, m),
                                    rhs=qt[:, c, :, :],
                                    start=(pi == 0 and c == 0),
                                    stop=(pi == 2 and c == KC2 - 1),
                                    perf_mode=DRM)

                def ev():
                    for m in ms:
                        qs = stage.tile([P, QROW], F32, tag="qstage",
                                        name=f"qs{m}")
                        nc.scalar.activation(qs[:], pss[m], RELU,
                                             bias=bq_sb[:, m:m + 1],
                                             scale=1.0 / SW)
                        qp = stage.tile([P, QROW], F32, tag="qstage",
                                        name=f"qp{m}")
                        nc.vector.tensor_mul(qp[:], qs[:], whx_sb)
                        qw = qwpool.tile([P, BS], F32, tag="qw", name=f"qw{m}")
                        nc.vector.tensor_reduce(
                            qw[:], qp.rearrange("p (b q) -> p b q", b=BS),
                            axis=mybir.AxisListType.X, op=mybir.AluOpType.add)
                        qwts[m] = qw

                return mm, ev

            # b0: B m0-7 before phase A; relu-evictions now (into persistent
            # bf16 stages), qw-muls deferred until A produces qw.
            b0_vs = {m: b0pool.tile([P, NROW], BF16, name=f"b0vs{m}")
                     for m in range(8)}
            for ms in ([0, 1, 2, 3], [4, 5, 6, 7]):
                for nb in range(NN):
                    pipe(*b_block(ms, nb, lambda m: b0_vs[m]))

            # phase A
            for blk in range(4):
                pipe(*a_block(list(range(blk * 4, blk * 4 + 4))))



            # phase B rest: per-m blocks (3 groups) so each m's logits
            # finish right after its last n-chunk, staggered ahead of C.
            # b0's 8 deferred finishes are spread two per block so they
            # don't burst-starve the eviction engines.
            b0_left = list(range(8))
            first = [True]
            for m in range(8, MH):
                vs = b0pool.tile([P, NROW], BF16, name=f"vs{m}")
                groups = [(m, nb) for nb in range(NN)]
                pss = {nb: ps_alloc(NT) for nb in range(NN)}

                def mm(m=m, pss=pss):
                    for pi, (wt, vt) in enumerate(
                            [(wvh, vh_s), (wvl, vh_s), (wvh, vl_s)]):
                        for c in range(KC1):
                            for nb in range(NN):
                                nc.tensor.matmul(
                                    pss[nb],
                                    lhsT=w_lhsT(wt, 512, c, m),
                                    rhs=vt[:, nb, c, :, :],
                                    start=(pi == 0 and c == 0),
                                    stop=(pi == 2 and c == KC1 - 1),
                                    perf_mode=DRM)

                def ev(m=m, pss=pss, vs=vs):
                    # relus first (PSUM banks recycle asap), then the two
                    # half-finishes so C's dependency chain stays short
                    for nb in range(NN):
                        nc.scalar.activation(
                            vs[:, nb * NT:(nb + 1) * NT], pss[nb],
                            RELU, bias=bv_sb[:, m:m + 1], scale=1.0 / SW)
                    b_finish(m, vs, half=0)
                    b_finish(m, vs, half=1)

                mm()
                if first[0]:
                    pipe_flush()
                    first[0] = False
                else:
                    _pending[0]()
                _pending[0] = ev
                for bm in b0_left[:1]:
                    b_finish(bm, b0_vs[bm], sub_eng=nc.gpsimd)
                b0_left = b0_left[1:]

            # ---- Phase C: out_T = (W2*SW).T @ [lts_hi/lo] / (SW*SL) + b2eff
            engs = {"sync": nc.sync, "scalar": nc.scalar,
                    "gpsimd": nc.gpsimd}
            out_engs = [engs[e] for e in o["out_engs"]]
            tail_engs = [engs[e] for e in o["tail_engs"]]
            CP = [(w2h, lts_hi), (w2l, lts_hi), (w2h, lts_lo)]

            def c_mm(ps, m, c0, w, cs, first, last):
                # (hh, lh, hl): logits-lo (produced last on-chip) is only
                # needed by the final pass
                for pi, (wt, lt) in enumerate(CP):
                    for ci, c in enumerate(cs):
                        nc.tensor.matmul(
                            ps, lhsT=w_lhsT(wt, 512, c, m),
                            rhs=lt[:, 2 * c:2 * c + 2, c0:c0 + w],
                            start=(first and pi == 0 and ci == 0),
                            stop=(last and pi == 2 and ci == len(cs) - 1),
                            perf_mode=DRM)

            # C head: the first two groups' c0-c5 steps only read logits
            # m0-11, so they run while B m15's eviction chain completes.
            os0 = ospool.tile([P, NROW], F32, tag="os", name="os0")
            headps = [ps_alloc(NT), ps_alloc(NT)]
            c_mm(headps[0], 0, 0, NT, range(6), True, False)
            c_mm(headps[1], 0, NT, NT, range(6), True, False)
            pipe_flush()   # B m15's eviction + logits finish
            c_mm(headps[0], 0, 0, NT, range(6, 8), False, True)
            c_mm(headps[1], 0, NT, NT, range(6, 8), False, True)

            def head_ev():
                for c0, ps in ((0, headps[0]), (NT, headps[1])):
                    nc.scalar.activation(os0[:, c0:c0 + NT], ps, IDENT,
                                         bias=b2_sb[:, 0:1],
                                         scale=1.0 / (SW * SL))
            _pending[0] = head_ev

            for m in range(MV):
                if m == 0:
                    os_ = os0
                    chunks = [(2 * NT, NT)]
                else:
                    os_ = ospool.tile([P, NROW], F32, tag="os", name=f"os{m}")
                    chunks = ([(0, NT), (NT, NT), (2 * NT, NT)]
                              if m < MV - 1
                              else [(0, 192), (192, 192), (384, 96),
                                    (480, 96)])
                last_m = (m == MV - 1)
                for i, (c0, w) in enumerate(chunks):
                    ps = ps_alloc(w)

                    def mm(ps=ps, m=m, c0=c0, w=w):
                        c_mm(ps, m, c0, w, range(KC3), True, True)

                    def ev(m=m, c0=c0, w=w, ps=ps, os_=os_, i=i,
                           last_m=last_m):
                        nc.scalar.activation(os_[:, c0:c0 + w], ps,
                                             IDENT, bias=b2_sb[:, m:m + 1],
                                             scale=1.0 / (SW * SL))
                        if last_m:
                            eng = tail_engs[i % len(tail_engs)]
                            eng.dma_start(
                                out=outT[m * P:(m + 1) * P, c0:c0 + w],
                                in_=os_[:, c0:c0 + w])
                        elif c0 + w == NROW:
                            eng = out_engs[m % len(out_engs)]
                            eng.dma_start(out=outT[m * P:(m + 1) * P, :],
                                          in_=os_[:])

                    pipe(mm, ev)
            pipe_flush()

    nc.compile()
    return nc


_NC_CACHE = {}


def get_program(opts=None):
    key = tuple(sorted(opts.items())) if opts else ()
    if key not in _NC_CACHE:
        _NC_CACHE[key] = _build_program(opts)
    return _NC_CACHE[key]


def _hilo(XT, scale, vchunks=None):
    """XT [K, C] f32 -> (hi, lo) [P, ...] fp8; k = c*256 + j*128 + p.

    vchunks: if set, output is n-chunk-major [P, NN, K//256, 2, NT].
    """
    s = np.asarray(XT, np.float32) * np.float32(scale)
    hi = s.astype(F8NP)
    lo = (s - hi.astype(np.float32)).astype(F8NP)

    def lay(x):
        K, C = x.shape
        y = x.reshape(K // 256, 2, P, C).transpose(2, 0, 1, 3)
        if vchunks:
            y = y.reshape(P, K // 256, 2, vchunks, C // vchunks)
            y = y.transpose(0, 3, 1, 2, 4)
        return np.ascontiguousarray(y).reshape(P, -1)

    return lay(hi), lay(lo)


def make_in_maps(v, q, Wv, bv, Wq, bq, wh, bh, W2, b2):
    """Host-side prep: shard batch, pre-transpose, quantize to fp8 hi/lo."""
    wvh, wvl = _hilo(Wv.astype(np.float32).T, SW)        # [VD,H] rows=k
    wqh, wql = _hilo(Wq.astype(np.float32).T, SW)        # [QD,H]
    w2h, w2l = _hilo(W2.astype(np.float32).T, SW)        # [H,VD]
    b2eff = (b2.astype(np.float64)
             + float(bh) * W2.astype(np.float64).sum(axis=1)).astype(np.float32)
    constC = np.zeros((P, 3 * 16 + QROW), np.float32)
    constC[:, 0:16] = bv.astype(np.float32).reshape(MH, P).T
    constC[:, 16:32] = bq.astype(np.float32).reshape(MH, P).T
    constC[:, 32:48] = b2eff.reshape(MV, P).T
    constC[:, 48:] = np.tile(wh.astype(np.float32) * np.float32(SL), BS)[None, :]

    shared = {
        "wvh": wvh, "wvl": wvl, "wqh": wqh, "wql": wql,
        "w2h": w2h, "w2l": w2l, "constC": constC,
    }
    in_maps = []
    for c in range(NCORES):
        b0 = c * BS
        v_sh = v[b0:b0 + BS].reshape(NROW, VD).astype(np.float32)
        q_sh = q[b0:b0 + BS].reshape(QROW, QD).astype(np.float32)
        vh, vl = _hilo(v_sh.T, 1.0, vchunks=NN)
        qh, ql = _hilo(q_sh.T, 1.0)
        m = dict(shared)
        m.update({"vh": vh, "vl": vl, "qh": qh, "ql": ql})
        in_maps.append(m)
    return in_maps


def assemble_output(results):
    outs = []
    for c in range(NCORES):
        outT = results[c]["outT"]                      # [VD, NROW] f32
        outs.append(np.ascontiguousarray(outT.T).reshape(BS, NO, VD))
    return np.concatenate(outs, axis=0)


def kernel(v, q, Wv, bv, Wq, bq, wh, bh, W2, b2, **_unused):
    v, q, Wv, bv, Wq, bq, wh, bh, W2, b2 = (
        np.asarray(x) for x in (v, q, Wv, bv, Wq, bq, wh, bh, W2, b2))
    nc = get_program()
    in_maps = make_in_maps(v, q, Wv, bv, Wq, bq, wh, bh, W2, b2)
    res = run_bass_kernel_spmd(nc, in_maps, list(range(NCORES)))
    return assemble_output(res.results)
